# revision 3
# baseline (speedup 1.0000x reference)
"""AntisymmetricRNN Trainium2 kernel — 8-core data-parallel over batch.

Math (per reference):
    mask = strictly-lower-tri; w_r = v_r * mask; A = w_r - w_r.T
    step:  h' = h + (1/TAU) * tanh( tanh(h) @ A + b_r - GAMMA*h )
           x_pred = tanh(h') @ w_o.T + b_o;   err_t = x_pred - x_t

Design (v3):
  * batch 256 sharded 8 ways (32 per core); recurrence local per core.
  * state layout "h-major": [128 partitions = h%128, free = (h//128, b)] so
    the recurrent matmul output lands in state layout -> zero transposes.
  * per step: 64 bf16 rec matmuls (lhsT = A 128x128 tiles, rhs = tanh(h)
    [128,32] slices) into a [128,256] PSUM bank, ping-ponged by step parity
    so next step's prewrite never waits on this step's reads.
  * elementwise in G=2 chunks of 128 free elems (vs v2's G=4 x 64): the ACT
    fixed cost is ~352 cycles/op, so fewer+bigger ops halve ACT queue time
    (4 ops x ~400ns = 1.6us/step vs v2's 8 x ~330 = 2.6us).
  * rec matmuls ordered [m0-3 x k0-3][m0-3 x k4-7][m4-7 x k0-3][m4-7 x k4-7]
    so bank c0 stops 16 MMs after th(c1) lands -> tanh chain starts early.
  * output projection runs EVERY step (8 matmuls, w_o^T moving, N=256) as
    deliberate PE filler during the tanh chain: keeps the tensor engine
    near-continuously busy so the HAM clock gate un-throttles to 8/8
    (the v2 trace showed K=4/8 — half PE clock — for ~the whole kernel).
    xp accumulates 4 steps into one [128,256] PSUM tile via tile_position
    rows, then one DVE subtract + one contiguous 128KB err DMA per group.
  * fully unrolled (no hardware loops).
"""

import numpy as np
import ml_dtypes
from contextlib import ExitStack

import concourse.bass as bass
import concourse.tile as tile
from concourse import mybir
from concourse.bass_utils import run_bass_kernel_spmd

# ---------------- problem constants (hardcoded per spec) ----------------
S, B, D, H = 512, 256, 256, 1024
NCORES = 8
BS = B // NCORES                  # 32 batch per core
TAU, GAMMA = 10.0, 0.1
INV_TAU = 1.0 / TAU
KT = H // 128                     # 8 contraction tiles
MT = H // 128                     # 8 output tiles
G = 2                             # elementwise chunks per step
CW = (MT // G) * BS               # chunk width in free elems (128)
MPQ = MT // G                     # m-tiles per chunk (4)
NSLOT = 4                         # xp accumulation slots per DMA group

TRACE = False                     # set True from test harness for profiling
LAST_RESULTS = None               # BassKernelResults stash for the harness

_BUILT = None


def _split_multi_waits(nc, max_waits: int = 1):
    """The walrus build here supports one sync-wait slot on CTRL-encoded
    instructions; split any multi-wait instruction's extra waits into a chain
    of preceding single-wait NOPs on the same engine (identical semantics)."""
    for fn in nc.m.functions:
        for bb in fn.blocks:
            new_insts = []
            for inst in bb.instructions:
                si = inst.sync_info
                if si is not None and len(si.on_wait) > max_waits:
                    waits = list(si.on_wait)
                    for w in waits[:-max_waits]:
                        nop = mybir.InstNoOp(
                            name=nc.get_next_instruction_name(), ins=[], outs=[])
                        nop.engine = inst.engine
                        nop.sync_info = mybir.SyncInfo(on_wait=[w], on_update=[])
                        nc.register_instruction(nop)
                        new_insts.append(nop)
                    si.on_wait = waits[-max_waits:]
                new_insts.append(inst)
            bb.instructions = new_insts


def _build_bass():
    nc = bass.Bass("TRN2", target_bir_lowering=False, debug=False,
                   num_devices=NCORES)
    dt = mybir.dt
    f32, bf16 = dt.float32, dt.bfloat16

    A_d = nc.dram_tensor("A", [128, KT * MT * 128], bf16, kind="ExternalInput").ap()
    Wo_d = nc.dram_tensor("Wo", [128, KT * D], bf16, kind="ExternalInput").ap()
    Br_d = nc.dram_tensor("Br", [128, MT * BS], f32, kind="ExternalInput").ap()
    h0_d = nc.dram_tensor("h0", [128, MT * BS], f32, kind="ExternalInput").ap()
    th0_d = nc.dram_tensor("th0", [128, MT * BS], bf16, kind="ExternalInput").ap()
    x_d = nc.dram_tensor("x", [S, BS, D], f32, kind="ExternalInput").ap()
    err_d = nc.dram_tensor("err", [S, BS, D], f32, kind="ExternalOutput").ap()

    Tanh = mybir.ActivationFunctionType.Tanh
    MUL, ADD, SUB = (mybir.AluOpType.mult, mybir.AluOpType.add,
                     mybir.AluOpType.subtract)

    # [S,BS,D] viewed as [S/4, (4*BS)=128, D]: one contiguous 128KB block per
    # 4-step group, partition = (step_low, b).
    x_g = x_d.rearrange("(g s) b d -> g (s b) d", s=NSLOT)
    e_g = err_d.rearrange("(g s) b d -> g (s b) d", s=NSLOT)

    with tile.TileContext(nc) as tc, ExitStack() as ctx:
        const = ctx.enter_context(tc.tile_pool(name="const", bufs=1))
        state = ctx.enter_context(tc.tile_pool(name="state", bufs=1))
        scratch = ctx.enter_context(tc.tile_pool(name="scratch", bufs=3))
        zpool = ctx.enter_context(tc.tile_pool(name="zps", bufs=1, space="PSUM"))
        xppool = ctx.enter_context(tc.tile_pool(name="xpps", bufs=2, space="PSUM"))
        xtp = ctx.enter_context(tc.tile_pool(name="xt", bufs=3))
        etp = ctx.enter_context(tc.tile_pool(name="et", bufs=3))

        A_sb = const.tile([128, KT * MT * 128], bf16, tag="A", name="A_sb")
        Wo_sb = const.tile([128, KT * D], bf16, tag="Wo", name="Wo_sb")
        Br_sb = const.tile([128, MT * BS], f32, tag="Br", name="Br_sb")
        nc.sync.dma_start(A_sb[:], A_d[:])
        nc.sync.dma_start(Wo_sb[:], Wo_d[:])
        nc.sync.dma_start(Br_sb[:], Br_d[:])

        hT = state.tile([128, MT * BS], f32, tag="h", name="hT")
        # tanh(h) ring: TH[j % 4] holds th after step j (rec of j+1 reads
        # slot (j)%4; proj of step j reads slot j%4).
        TH = [state.tile([128, MT * BS], bf16, tag=f"TH{s}", name=f"TH{s}")
              for s in range(NSLOT)]
        # s-accumulator PSUM banks, ping-pong by step parity.
        zT = [zpool.tile([128, MT * BS], f32, tag=f"z{p}", name=f"zT{p}")
              for p in range(2)]
        zeros = const.tile([128, MT * BS], bf16, tag="zeros", name="zeros")
        nc.vector.memset(zeros[:], 0.0)
        nc.sync.dma_start(hT[:], h0_d[:])
        nc.sync.dma_start(TH[NSLOT - 1][:], th0_d[:])

        # Prime PSUM has_written bits with a zero matmul, then pre-write
        # t = b_r - GAMMA*h into the bank; every step's matmuls accumulate
        # on top (start=False), so the bank holds s = th@A + b_r - GAMMA*h
        # when its k-loop finishes.
        def emit_prewrite(par, c):
            sl = slice(c * CW, (c + 1) * CW)
            nc.vector.scalar_tensor_tensor(
                zT[par][:, sl], hT[:, sl], -GAMMA, Br_sb[:, sl], MUL, ADD)
        for p in range(2):
            nc.tensor.matmul(zT[p][:], lhsT=zeros[:, :128], rhs=zeros[:],
                             start=True, stop=True)
        for c in range(G):
            emit_prewrite(0, c)

        def emit_rec(j):
            """64 matmuls of step j: zT[j%2] += A^T @ th_{j-1} tiles.
            Order: [m0-3 x k0-3][m0-3 x k4-7][m4-7 x k0-3][m4-7 x k4-7] so
            bank chunk c0 stops just 16 MMs after th(c1) of step j-1 lands."""
            par = j % 2
            rd = (j - 1) % NSLOT
            z = zT[par]
            for mg, kg in ((0, 0), (0, 1), (1, 0), (1, 1)):
                for m in range(mg * MPQ, (mg + 1) * MPQ):
                    for k in range(kg * MPQ, (kg + 1) * MPQ):
                        nc.tensor.matmul(
                            z[:, m * BS:(m + 1) * BS],
                            lhsT=A_sb[:, (k * MT + m) * 128:(k * MT + m + 1) * 128],
                            rhs=TH[rd][:, k * BS:(k + 1) * BS],
                            start=False, stop=(k == KT - 1),
                            skip_group_check=True)

        def emit_chain(j):
            """Per-chunk tanh/update chain for step j (z bank j%2).
            Emission order puts u0,u1 ahead of th0,th1 on the ACT queue so
            u1 (input ready early) never queues behind th0 (waiting on DVE)."""
            par, wr = j % 2, j % NSLOT
            u_t = []
            for c in range(G):
                sl = slice(c * CW, (c + 1) * CW)
                u = scratch.tile([128, CW], f32, tag="u", name="u_t")
                nc.scalar.activation(u[:], zT[par][:, sl], Tanh)    # u=tanh(s)
                nc.vector.scalar_tensor_tensor(                     # h += u/TAU
                    hT[:, sl], u[:], INV_TAU, hT[:, sl], MUL, ADD)
                u_t.append(u)
            for c in range(G):
                sl = slice(c * CW, (c + 1) * CW)
                nc.scalar.activation(TH[wr][:, sl], hT[:, sl], Tanh)
                if j < S - 1:
                    emit_prewrite(1 - par, c)                       # t for j+1

        def emit_proj(j, xp):
            """x_pred for step j: xp[32*(j%4)+b, d] += th_j[k][:,b]^T @ w_o^T.
            N=256 moving operand; doubles as PE filler while the tanh chain
            of step j+1 runs."""
            s = j % NSLOT
            rd = j % NSLOT
            for k in range(KT):
                nc.tensor.matmul(
                    xp[32 * s:32 * (s + 1), :],
                    lhsT=TH[rd][:, k * BS:(k + 1) * BS],
                    rhs=Wo_sb[:, k * D:(k + 1) * D],
                    start=(k == 0), stop=(k == KT - 1),
                    tile_position=(0, 32 * s))

        def emit_group_out(g, xp):
            xt = xtp.tile([128, D], f32, tag="xt", name="xt")
            nc.sync.dma_start(xt[:], x_g[g])
            et = etp.tile([128, D], f32, tag="et", name="et")
            nc.vector.scalar_tensor_tensor(                        # xp-(x-b_o)
                et[:], xp[:], 0.0, xt[:], ADD, SUB)
            nc.sync.dma_start(e_g[g], et[:])

        xp = None
        for j in range(S):
            emit_rec(j)
            emit_chain(j)
            # proj of the PREVIOUS step as filler while this step's chain runs
            if j > 0:
                if (j - 1) % NSLOT == 0:
                    xp = xppool.tile([128, D], f32, tag="xp", name="xp")
                emit_proj(j - 1, xp)
                if (j - 1) % NSLOT == NSLOT - 1:
                    emit_group_out((j - 1) // NSLOT, xp)
        emit_proj(S - 1, xp)
        emit_group_out((S - 1) // NSLOT, xp)

    _split_multi_waits(nc)
    return nc


def _host_prep(x, h_init, v_r, b_r, w_o, b_o):
    """Build per-core input maps (all layout work in numpy)."""
    x = np.asarray(x, np.float32)
    h_init = np.asarray(h_init, np.float32)
    v_r = np.asarray(v_r, np.float32)
    b_r = np.asarray(b_r, np.float32)
    w_o = np.asarray(w_o, np.float32)
    b_o = np.asarray(b_o, np.float32)

    mask = np.tril(np.ones((H, H), np.float32), -1)
    w_r = v_r * mask
    A = w_r - w_r.T                                           # [H, H]
    # A_sb[p, (k*MT+m)*128 + c] = A[k*128+p, m*128+c]
    A_sb = np.ascontiguousarray(
        A.reshape(KT, 128, MT, 128).transpose(1, 0, 2, 3).reshape(128, KT * MT * 128)
    ).astype(ml_dtypes.bfloat16)
    # Wo_sb[p, k*D + d] = w_o[d, k*128+p]   (w_o^T tiles, moving operand)
    Wo_sb = np.ascontiguousarray(
        w_o.T.reshape(KT, 128, D).transpose(1, 0, 2).reshape(128, KT * D)
    ).astype(ml_dtypes.bfloat16)
    # Br[p, m*BS+b] = b_r[m*128+p]
    Br = np.ascontiguousarray(
        np.broadcast_to(b_r.reshape(MT, 128, 1).transpose(1, 0, 2), (128, MT, BS))
    ).reshape(128, MT * BS).astype(np.float32)

    in_maps = []
    for c in range(NCORES):
        hc = h_init[c * BS:(c + 1) * BS]                       # [BS, H]
        h0 = np.ascontiguousarray(
            hc.reshape(BS, MT, 128).transpose(2, 1, 0)         # [128, MT, BS]
        ).reshape(128, MT * BS).astype(np.float32)
        th0 = np.tanh(h0)
        in_maps.append({
            "A": A_sb, "Wo": Wo_sb, "Br": Br,
            "h0": h0, "th0": th0.astype(ml_dtypes.bfloat16),
            "x": np.ascontiguousarray(x[:, c * BS:(c + 1) * BS, :] - b_o),
        })
    return in_maps


def kernel(x, h_init, v_r, b_r, w_o, b_o):
    global _BUILT, LAST_RESULTS
    if _BUILT is None:
        _BUILT = _build_bass()
    nc = _BUILT
    in_maps = _host_prep(x, h_init, v_r, b_r, w_o, b_o)
    res = run_bass_kernel_spmd(nc, in_maps, core_ids=list(range(NCORES)),
                               trace=TRACE)
    LAST_RESULTS = res
    out = np.empty((S, B, D), np.float32)
    for c in range(NCORES):
        out[:, c * BS:(c + 1) * BS, :] = np.asarray(res.results[c]["err"])
    return out


# revision 10
# speedup vs baseline: 1.0721x; 1.0721x over previous
"""AntisymmetricRNN Trainium2 kernel — 8-core data-parallel over batch.

Math (per reference):
    mask = strictly-lower-tri; w_r = v_r * mask; A = w_r - w_r.T
    step:  h' = h + (1/TAU) * tanh( tanh(h) @ A + b_r - GAMMA*h )
           x_pred = tanh(h') @ w_o.T + b_o;   err_t = x_pred - x_t

Design (v3):
  * batch 256 sharded 8 ways (32 per core); recurrence local per core.
  * state layout "h-major": [128 partitions = h%128, free = (h//128, b)] so
    the recurrent matmul output lands in state layout -> zero transposes.
  * per step: 64 bf16 rec matmuls (lhsT = A 128x128 tiles, rhs = tanh(h)
    [128,32] slices) into a [128,256] PSUM bank, ping-ponged by step parity
    so next step's prewrite never waits on this step's reads.
  * elementwise in G=2 chunks of 128 free elems (vs v2's G=4 x 64): the ACT
    fixed cost is ~352 cycles/op, so fewer+bigger ops halve ACT queue time
    (4 ops x ~400ns = 1.6us/step vs v2's 8 x ~330 = 2.6us).
  * rec matmuls ordered [m0-3 x k0-3][m0-3 x k4-7][m4-7 x k0-3][m4-7 x k4-7]
    so bank c0 stops 16 MMs after th(c1) lands -> tanh chain starts early.
  * output projection runs EVERY step (8 matmuls, w_o^T moving, N=256) as
    deliberate PE filler during the tanh chain: keeps the tensor engine
    near-continuously busy so the HAM clock gate un-throttles to 8/8
    (the v2 trace showed K=4/8 — half PE clock — for ~the whole kernel).
    xp accumulates 4 steps into one [128,256] PSUM tile via tile_position
    rows, then one DVE subtract + one contiguous 128KB err DMA per group.
  * fully unrolled (no hardware loops).
"""

import numpy as np
import ml_dtypes
from contextlib import ExitStack

import concourse.bass as bass
import concourse.tile as tile
from concourse import mybir
from concourse.bass_utils import run_bass_kernel_spmd

# ---------------- problem constants (hardcoded per spec) ----------------
S, B, D, H = 512, 256, 256, 1024
NCORES = 8
BS = B // NCORES                  # 32 batch per core
TAU, GAMMA = 10.0, 0.1
INV_TAU = 1.0 / TAU
KT = H // 128                     # 8 contraction tiles
MT = H // 128                     # 8 output tiles
G = 2                             # elementwise chunks per step
CW = (MT // G) * BS               # chunk width in free elems (128)
MPQ = MT // G                     # m-tiles per chunk (4)
NSLOT = 4                         # xp accumulation slots per DMA group
NRING = 5                         # tanh(h) ring depth: 4 proj slots + 1 so
                                  # the proj group never WAR-stalls the chain

TRACE = False                     # set True from test harness for profiling
LAST_RESULTS = None               # BassKernelResults stash for the harness

_BUILT = None


def _split_multi_waits(nc, max_waits: int = 1):
    """The walrus build here supports one sync-wait slot on CTRL-encoded
    instructions; split any multi-wait instruction's extra waits into a chain
    of preceding single-wait NOPs on the same engine (identical semantics)."""
    for fn in nc.m.functions:
        for bb in fn.blocks:
            new_insts = []
            for inst in bb.instructions:
                si = inst.sync_info
                if si is not None and len(si.on_wait) > max_waits:
                    waits = list(si.on_wait)
                    for w in waits[:-max_waits]:
                        nop = mybir.InstNoOp(
                            name=nc.get_next_instruction_name(), ins=[], outs=[])
                        nop.engine = inst.engine
                        nop.sync_info = mybir.SyncInfo(on_wait=[w], on_update=[])
                        nc.register_instruction(nop)
                        new_insts.append(nop)
                    si.on_wait = waits[-max_waits:]
                new_insts.append(inst)
            bb.instructions = new_insts


def _build_bass():
    nc = bass.Bass("TRN2", target_bir_lowering=False, debug=False,
                   num_devices=NCORES)
    dt = mybir.dt
    f32, bf16 = dt.float32, dt.bfloat16

    A_d = nc.dram_tensor("A", [128, KT * MT * 128], bf16, kind="ExternalInput").ap()
    Wo_d = nc.dram_tensor("Wo", [128, KT * D], bf16, kind="ExternalInput").ap()
    Br_d = nc.dram_tensor("Br", [128, MT * BS], f32, kind="ExternalInput").ap()
    h0_d = nc.dram_tensor("h0", [128, MT * BS], f32, kind="ExternalInput").ap()
    th0_d = nc.dram_tensor("th0", [128, MT * BS], bf16, kind="ExternalInput").ap()
    x_d = nc.dram_tensor("x", [S, BS, D], f32, kind="ExternalInput").ap()
    err_d = nc.dram_tensor("err", [S, BS, D], f32, kind="ExternalOutput").ap()

    Tanh = mybir.ActivationFunctionType.Tanh
    MUL, ADD, SUB = (mybir.AluOpType.mult, mybir.AluOpType.add,
                     mybir.AluOpType.subtract)

    # [S,BS,D] viewed as [S/4, (4*BS)=128, D]: one contiguous 128KB block per
    # 4-step group, partition = (step_low, b).
    x_g = x_d.rearrange("(g s) b d -> g (s b) d", s=NSLOT)
    e_g = err_d.rearrange("(g s) b d -> g (s b) d", s=NSLOT)

    with tile.TileContext(nc) as tc, ExitStack() as ctx:
        const = ctx.enter_context(tc.tile_pool(name="const", bufs=1))
        state = ctx.enter_context(tc.tile_pool(name="state", bufs=1))
        scratch = ctx.enter_context(tc.tile_pool(name="scratch", bufs=3))
        zpool = ctx.enter_context(tc.tile_pool(name="zps", bufs=1, space="PSUM"))
        xppool = ctx.enter_context(tc.tile_pool(name="xpps", bufs=2, space="PSUM"))
        xtp = ctx.enter_context(tc.tile_pool(name="xt", bufs=3))
        etp = ctx.enter_context(tc.tile_pool(name="et", bufs=3))

        A_sb = const.tile([128, KT * MT * 128], bf16, tag="A", name="A_sb")
        Wo_sb = const.tile([128, KT * D], bf16, tag="Wo", name="Wo_sb")
        Br_sb = const.tile([128, MT * BS], f32, tag="Br", name="Br_sb")
        nc.sync.dma_start(A_sb[:], A_d[:])
        nc.sync.dma_start(Wo_sb[:], Wo_d[:])
        nc.sync.dma_start(Br_sb[:], Br_d[:])

        hT = state.tile([128, MT * BS], f32, tag="h", name="hT")
        # tanh(h) ring: TH[j % NRING] holds th after step j.
        TH = [state.tile([128, MT * BS], bf16, tag=f"TH{s}", name=f"TH{s}")
              for s in range(NRING)]
        # s-accumulator PSUM banks, ping-pong by step parity.
        zT = [zpool.tile([128, MT * BS], f32, tag=f"z{p}", name=f"zT{p}")
              for p in range(2)]
        zeros = const.tile([128, MT * BS], bf16, tag="zeros", name="zeros")
        nc.vector.memset(zeros[:], 0.0)
        nc.sync.dma_start(hT[:], h0_d[:])
        nc.sync.dma_start(TH[NRING - 1][:], th0_d[:])

        # Prime PSUM has_written bits with a zero matmul, then pre-write
        # t = b_r - GAMMA*h into the bank; every step's matmuls accumulate
        # on top (start=False), so the bank holds s = th@A + b_r - GAMMA*h
        # when its k-loop finishes.
        def emit_prewrite(par, c):
            sl = slice(c * CW, (c + 1) * CW)
            nc.vector.scalar_tensor_tensor(
                zT[par][:, sl], hT[:, sl], -GAMMA, Br_sb[:, sl], MUL, ADD)
        for p in range(2):
            nc.tensor.matmul(zT[p][:], lhsT=zeros[:, :128], rhs=zeros[:],
                             start=True, stop=True)
        for c in range(G):
            emit_prewrite(0, c)

        def emit_rec(j):
            """64 matmuls of step j: zT[j%2] += A^T @ th_{j-1} tiles.
            Order: [m0-3 x k0-3][m0-3 x k4-7][m4-7 x k0-3][m4-7 x k4-7] so
            bank chunk c0 stops just 16 MMs after th(c1) of step j-1 lands."""
            par = j % 2
            rd = (j - 1) % NRING
            z = zT[par]
            for mg, kg in ((0, 0), (0, 1), (1, 0), (1, 1)):
                for m in range(mg * MPQ, (mg + 1) * MPQ):
                    for k in range(kg * MPQ, (kg + 1) * MPQ):
                        nc.tensor.matmul(
                            z[:, m * BS:(m + 1) * BS],
                            lhsT=A_sb[:, (k * MT + m) * 128:(k * MT + m + 1) * 128],
                            rhs=TH[rd][:, k * BS:(k + 1) * BS],
                            start=False, stop=(k == KT - 1),
                            skip_group_check=True)

        def emit_chain(j):
            """Per-chunk tanh/update chain for step j (z bank j%2).
            Emission order puts u0,u1 ahead of th0,th1 on the ACT queue so
            u1 (input ready early) never queues behind th0 (waiting on DVE)."""
            par, wr = j % 2, j % NRING
            u_t = []
            for c in range(G):
                sl = slice(c * CW, (c + 1) * CW)
                u = scratch.tile([128, CW], f32, tag="u", name="u_t")
                nc.scalar.activation(u[:], zT[par][:, sl], Tanh)    # u=tanh(s)
                nc.vector.scalar_tensor_tensor(                     # h += u/TAU
                    hT[:, sl], u[:], INV_TAU, hT[:, sl], MUL, ADD)
                u_t.append(u)
            for c in range(G):
                sl = slice(c * CW, (c + 1) * CW)
                nc.scalar.activation(TH[wr][:, sl], hT[:, sl], Tanh)
                if j < S - 1:
                    emit_prewrite(1 - par, c)                       # t for j+1

        def emit_proj_group(g):
            """x_pred for steps 4g..4g+3: per k-tile, 4 column-tiled matmuls
            (one per slot, tile_position col 32s) share one 256-col w_o^T
            stream — ~4x cheaper than slot-sequential.  Runs as end-of-step
            PE filler during step 4g+4's tanh chain."""
            xp = xppool.tile([128, D], f32, tag="xp", name="xp")
            for k in range(KT):
                for s in range(NSLOT):
                    nc.tensor.matmul(
                        xp[32 * s:32 * (s + 1), :],
                        lhsT=TH[(4 * g + s) % NRING][:, k * BS:(k + 1) * BS],
                        rhs=Wo_sb[:, k * D:(k + 1) * D],
                        start=(k == 0), stop=(k == KT - 1),
                        tile_position=(0, 32 * s))
            return xp

        def emit_group_out(g, xp):
            et = etp.tile([128, D], f32, tag="et", name="et")
            nc.vector.scalar_tensor_tensor(                        # xp-(x-b_o)
                et[:], xp[:], 0.0, xt_tiles.pop(g)[:], ADD, SUB)
            nc.sync.dma_start(e_g[g], et[:])

        xt_tiles = {}

        def prefetch_x(g):
            xt = xtp.tile([128, D], f32, tag="xt", name="xt")
            nc.sync.dma_start(xt[:], x_g[g])
            xt_tiles[g] = xt

        prefetch_x(0)
        xp = None
        for j in range(S):
            emit_rec(j)
            if j % NSLOT == 0 and j > 0:
                xp = emit_proj_group(j // NSLOT - 1)
            emit_chain(j)
            if j % NSLOT == 1 and j // NSLOT + 1 <= S // NSLOT - 1:
                prefetch_x(j // NSLOT + 1)
            if j % NSLOT == 0 and j > 0:
                emit_group_out(j // NSLOT - 1, xp)
        xp = emit_proj_group(S // NSLOT - 1)
        emit_group_out(S // NSLOT - 1, xp)

    _split_multi_waits(nc)
    return nc


def _host_prep(x, h_init, v_r, b_r, w_o, b_o):
    """Build per-core input maps (all layout work in numpy)."""
    x = np.asarray(x, np.float32)
    h_init = np.asarray(h_init, np.float32)
    v_r = np.asarray(v_r, np.float32)
    b_r = np.asarray(b_r, np.float32)
    w_o = np.asarray(w_o, np.float32)
    b_o = np.asarray(b_o, np.float32)

    mask = np.tril(np.ones((H, H), np.float32), -1)
    w_r = v_r * mask
    A = w_r - w_r.T                                           # [H, H]
    # A_sb[p, (k*MT+m)*128 + c] = A[k*128+p, m*128+c]
    A_sb = np.ascontiguousarray(
        A.reshape(KT, 128, MT, 128).transpose(1, 0, 2, 3).reshape(128, KT * MT * 128)
    ).astype(ml_dtypes.bfloat16)
    # Wo_sb[p, k*D + d] = w_o[d, k*128+p]   (w_o^T tiles, moving operand)
    Wo_sb = np.ascontiguousarray(
        w_o.T.reshape(KT, 128, D).transpose(1, 0, 2).reshape(128, KT * D)
    ).astype(ml_dtypes.bfloat16)
    # Br[p, m*BS+b] = b_r[m*128+p]
    Br = np.ascontiguousarray(
        np.broadcast_to(b_r.reshape(MT, 128, 1).transpose(1, 0, 2), (128, MT, BS))
    ).reshape(128, MT * BS).astype(np.float32)

    in_maps = []
    for c in range(NCORES):
        hc = h_init[c * BS:(c + 1) * BS]                       # [BS, H]
        h0 = np.ascontiguousarray(
            hc.reshape(BS, MT, 128).transpose(2, 1, 0)         # [128, MT, BS]
        ).reshape(128, MT * BS).astype(np.float32)
        th0 = np.tanh(h0)
        in_maps.append({
            "A": A_sb, "Wo": Wo_sb, "Br": Br,
            "h0": h0, "th0": th0.astype(ml_dtypes.bfloat16),
            "x": np.ascontiguousarray(x[:, c * BS:(c + 1) * BS, :] - b_o),
        })
    return in_maps


def kernel(x, h_init, v_r, b_r, w_o, b_o):
    global _BUILT, LAST_RESULTS
    if _BUILT is None:
        _BUILT = _build_bass()
    nc = _BUILT
    in_maps = _host_prep(x, h_init, v_r, b_r, w_o, b_o)
    res = run_bass_kernel_spmd(nc, in_maps, core_ids=list(range(NCORES)),
                               trace=TRACE)
    LAST_RESULTS = res
    out = np.empty((S, B, D), np.float32)
    for c in range(NCORES):
        out[:, c * BS:(c + 1) * BS, :] = np.asarray(res.results[c]["err"])
    return out


# revision 12
# speedup vs baseline: 1.2645x; 1.1795x over previous
"""AntisymmetricRNN Trainium2 kernel — 8-core data-parallel over batch.

Math (per reference):
    mask = strictly-lower-tri; w_r = v_r * mask; A = w_r - w_r.T
    step:  h' = h + (1/TAU) * tanh( tanh(h) @ A + b_r - GAMMA*h )
           x_pred = tanh(h') @ w_o.T + b_o;   err_t = x_pred - x_t

Design (v3):
  * batch 256 sharded 8 ways (32 per core); recurrence local per core.
  * state layout "h-major": [128 partitions = h%128, free = (h//128, b)] so
    the recurrent matmul output lands in state layout -> zero transposes.
  * per step: 64 bf16 rec matmuls (lhsT = A 128x128 tiles, rhs = tanh(h)
    [128,32] slices) into a [128,256] PSUM bank, ping-ponged by step parity
    so next step's prewrite never waits on this step's reads.
  * elementwise in G=2 chunks of 128 free elems (vs v2's G=4 x 64): the ACT
    fixed cost is ~352 cycles/op, so fewer+bigger ops halve ACT queue time
    (4 ops x ~400ns = 1.6us/step vs v2's 8 x ~330 = 2.6us).
  * rec matmuls ordered [m0-3 x k0-3][m0-3 x k4-7][m4-7 x k0-3][m4-7 x k4-7]
    so bank c0 stops 16 MMs after th(c1) lands -> tanh chain starts early.
  * output projection runs EVERY step (8 matmuls, w_o^T moving, N=256) as
    deliberate PE filler during the tanh chain: keeps the tensor engine
    near-continuously busy so the HAM clock gate un-throttles to 8/8
    (the v2 trace showed K=4/8 — half PE clock — for ~the whole kernel).
    xp accumulates 4 steps into one [128,256] PSUM tile via tile_position
    rows, then one DVE subtract + one contiguous 128KB err DMA per group.
  * fully unrolled (no hardware loops).
"""

import numpy as np
import ml_dtypes
from contextlib import ExitStack

import concourse.bass as bass
import concourse.tile as tile
from concourse import mybir
from concourse.bass_utils import run_bass_kernel_spmd

# ---------------- problem constants (hardcoded per spec) ----------------
S, B, D, H = 512, 256, 256, 1024
NCORES = 8
BS = B // NCORES                  # 32 batch per core
TAU, GAMMA = 10.0, 0.1
INV_TAU = 1.0 / TAU
KT = H // 128                     # 8 contraction tiles
MT = H // 128                     # 8 output tiles
G = 2                             # elementwise chunks per step
CW = (MT // G) * BS               # chunk width in free elems (128)
MPQ = MT // G                     # m-tiles per chunk (4)
NSLOT = 4                         # xp accumulation slots per DMA group
NRING = 5                         # tanh(h) ring depth: 4 proj slots + 1 so
                                  # the proj group never WAR-stalls the chain

TRACE = False                     # set True from test harness for profiling
LAST_RESULTS = None               # BassKernelResults stash for the harness

_BUILT = None


def _split_multi_waits(nc, max_waits: int = 1):
    """The walrus build here supports one sync-wait slot on CTRL-encoded
    instructions; split any multi-wait instruction's extra waits into a chain
    of preceding single-wait NOPs on the same engine (identical semantics)."""
    for fn in nc.m.functions:
        for bb in fn.blocks:
            new_insts = []
            for inst in bb.instructions:
                si = inst.sync_info
                if si is not None and len(si.on_wait) > max_waits:
                    waits = list(si.on_wait)
                    for w in waits[:-max_waits]:
                        nop = mybir.InstNoOp(
                            name=nc.get_next_instruction_name(), ins=[], outs=[])
                        nop.engine = inst.engine
                        nop.sync_info = mybir.SyncInfo(on_wait=[w], on_update=[])
                        nc.register_instruction(nop)
                        new_insts.append(nop)
                    si.on_wait = waits[-max_waits:]
                new_insts.append(inst)
            bb.instructions = new_insts


def _build_bass():
    nc = bass.Bass("TRN2", target_bir_lowering=False, debug=False,
                   num_devices=NCORES)
    dt = mybir.dt
    f32, bf16 = dt.float32, dt.bfloat16

    A_d = nc.dram_tensor("A", [128, KT * MT * 128], bf16, kind="ExternalInput").ap()
    Wo_d = nc.dram_tensor("Wo", [128, KT * D], bf16, kind="ExternalInput").ap()
    Br_d = nc.dram_tensor("Br", [128, MT * BS], f32, kind="ExternalInput").ap()
    h0_d = nc.dram_tensor("h0", [128, MT * BS], f32, kind="ExternalInput").ap()
    th0_d = nc.dram_tensor("th0", [128, MT * BS], bf16, kind="ExternalInput").ap()
    x_d = nc.dram_tensor("x", [S, BS, D], f32, kind="ExternalInput").ap()
    err_d = nc.dram_tensor("err", [S, BS, D], f32, kind="ExternalOutput").ap()

    Tanh = mybir.ActivationFunctionType.Tanh
    MUL, ADD, SUB = (mybir.AluOpType.mult, mybir.AluOpType.add,
                     mybir.AluOpType.subtract)

    # [S,BS,D] viewed as [S/4, (4*BS)=128, D]: one contiguous 128KB block per
    # 4-step group, partition = (step_low, b).
    x_g = x_d.rearrange("(g s) b d -> g (s b) d", s=NSLOT)
    e_g = err_d.rearrange("(g s) b d -> g (s b) d", s=NSLOT)

    with tile.TileContext(nc) as tc, ExitStack() as ctx:
        const = ctx.enter_context(tc.tile_pool(name="const", bufs=1))
        state = ctx.enter_context(tc.tile_pool(name="state", bufs=1))
        scratch = ctx.enter_context(tc.tile_pool(name="scratch", bufs=3))
        zpool = ctx.enter_context(tc.tile_pool(name="zps", bufs=1, space="PSUM"))
        xppool = ctx.enter_context(tc.tile_pool(name="xpps", bufs=2, space="PSUM"))
        xtp = ctx.enter_context(tc.tile_pool(name="xt", bufs=3))
        etp = ctx.enter_context(tc.tile_pool(name="et", bufs=3))

        A_sb = const.tile([128, KT * MT * 128], bf16, tag="A", name="A_sb")
        Wo_sb = const.tile([128, KT * D], bf16, tag="Wo", name="Wo_sb")
        Br_sb = const.tile([128, MT * BS], f32, tag="Br", name="Br_sb")
        nc.sync.dma_start(A_sb[:], A_d[:])
        nc.sync.dma_start(Wo_sb[:], Wo_d[:])
        nc.sync.dma_start(Br_sb[:], Br_d[:])

        # Per-chunk tiles (NOT slices of one big tile): the tile framework
        # tracks deps at tile granularity, so chunk-level tiles are what let
        # the tanh chain of chunk c0 start while chunk c1's matmuls stream.
        hT = [state.tile([128, CW], f32, tag=f"h{c}", name=f"hT{c}")
              for c in range(G)]
        # tanh(h) ring: TH[j % NRING][c] holds th chunk c after step j.
        TH = [[state.tile([128, CW], bf16, tag=f"TH{s}_{c}", name=f"TH{s}_{c}")
               for c in range(G)] for s in range(NRING)]
        # s-accumulator PSUM tiles, ping-pong by step parity, per chunk.
        zT = [[zpool.tile([128, CW], f32, tag=f"z{p}_{c}", name=f"zT{p}_{c}")
               for c in range(G)] for p in range(2)]
        zeros = const.tile([128, CW], bf16, tag="zeros", name="zeros")
        nc.vector.memset(zeros[:], 0.0)
        for c in range(G):
            sl = slice(c * CW, (c + 1) * CW)
            nc.sync.dma_start(hT[c][:], h0_d[:, sl])
            nc.sync.dma_start(TH[NRING - 1][c][:], th0_d[:, sl])

        # Prime PSUM has_written bits with a zero matmul, then pre-write
        # t = b_r - GAMMA*h into the bank; every step's matmuls accumulate
        # on top (start=False), so the bank holds s = th@A + b_r - GAMMA*h
        # when its k-loop finishes.
        def emit_prewrite(par, c):
            nc.vector.scalar_tensor_tensor(
                zT[par][c][:], hT[c][:], -GAMMA,
                Br_sb[:, c * CW:(c + 1) * CW], MUL, ADD)
        for p in range(2):
            for c in range(G):
                nc.tensor.matmul(zT[p][c][:], lhsT=zeros[:, :128],
                                 rhs=zeros[:], start=True, stop=True)
        for c in range(G):
            emit_prewrite(0, c)

        def th_slice(s, k):
            return TH[s][k // MPQ][:, (k % MPQ) * BS:((k % MPQ) + 1) * BS]

        def emit_rec(j):
            """64 matmuls of step j: zT[j%2] += A^T @ th_{j-1} tiles.
            Order: [m0-3 x k0-3][m0-3 x k4-7][m4-7 x k0-3][m4-7 x k4-7] so
            bank chunk c0 stops just 16 MMs after th(c1) of step j-1 lands."""
            par = j % 2
            rd = (j - 1) % NRING
            for mg, kg in ((0, 0), (0, 1), (1, 0), (1, 1)):
                for m in range(mg * MPQ, (mg + 1) * MPQ):
                    z = zT[par][m // MPQ]
                    mo = m % MPQ
                    for k in range(kg * MPQ, (kg + 1) * MPQ):
                        nc.tensor.matmul(
                            z[:, mo * BS:(mo + 1) * BS],
                            lhsT=A_sb[:, (k * MT + m) * 128:(k * MT + m + 1) * 128],
                            rhs=th_slice(rd, k),
                            start=False, stop=(k == KT - 1),
                            skip_group_check=True)

        def emit_chain(j):
            """Per-chunk tanh/update chain for step j (z bank j%2).
            Emission order puts u0,u1 ahead of th0,th1 on the ACT queue so
            u1 (input ready early) never queues behind th0 (waiting on DVE)."""
            par, wr = j % 2, j % NRING
            u_t = []
            for c in range(G):
                u = scratch.tile([128, CW], f32, tag="u", name="u_t")
                nc.scalar.activation(u[:], zT[par][c][:], Tanh)     # u=tanh(s)
                nc.vector.scalar_tensor_tensor(                     # h += u/TAU
                    hT[c][:], u[:], INV_TAU, hT[c][:], MUL, ADD)
                u_t.append(u)
            for c in range(G):
                nc.scalar.activation(TH[wr][c][:], hT[c][:], Tanh)
                if j < S - 1:
                    emit_prewrite(1 - par, c)                       # t for j+1

        def emit_proj_group(g):
            """x_pred for steps 4g..4g+3: per k-tile, 4 column-tiled matmuls
            (one per slot, tile_position col 32s) share one 256-col w_o^T
            stream — ~4x cheaper than slot-sequential.  Runs as end-of-step
            PE filler during step 4g+4's tanh chain."""
            xp = xppool.tile([128, D], f32, tag="xp", name="xp")
            for k in range(KT):
                for s in range(NSLOT):
                    nc.tensor.matmul(
                        xp[32 * s:32 * (s + 1), :],
                        lhsT=th_slice((4 * g + s) % NRING, k),
                        rhs=Wo_sb[:, k * D:(k + 1) * D],
                        start=(k == 0), stop=(k == KT - 1),
                        tile_position=(0, 32 * s))
            return xp

        def emit_group_out(g, xp):
            et = etp.tile([128, D], f32, tag="et", name="et")
            nc.vector.scalar_tensor_tensor(                        # xp-(x-b_o)
                et[:], xp[:], 0.0, xt_tiles.pop(g)[:], ADD, SUB)
            nc.sync.dma_start(e_g[g], et[:])

        xt_tiles = {}

        def prefetch_x(g):
            xt = xtp.tile([128, D], f32, tag="xt", name="xt")
            nc.sync.dma_start(xt[:], x_g[g])
            xt_tiles[g] = xt

        prefetch_x(0)
        xp = None
        for j in range(S):
            emit_rec(j)
            if j % NSLOT == 0 and j > 0:
                xp = emit_proj_group(j // NSLOT - 1)
            emit_chain(j)
            if j % NSLOT == 1 and j // NSLOT + 1 <= S // NSLOT - 1:
                prefetch_x(j // NSLOT + 1)
            if j % NSLOT == 0 and j > 0:
                emit_group_out(j // NSLOT - 1, xp)
        xp = emit_proj_group(S // NSLOT - 1)
        emit_group_out(S // NSLOT - 1, xp)

    _split_multi_waits(nc)
    return nc


def _host_prep(x, h_init, v_r, b_r, w_o, b_o):
    """Build per-core input maps (all layout work in numpy)."""
    x = np.asarray(x, np.float32)
    h_init = np.asarray(h_init, np.float32)
    v_r = np.asarray(v_r, np.float32)
    b_r = np.asarray(b_r, np.float32)
    w_o = np.asarray(w_o, np.float32)
    b_o = np.asarray(b_o, np.float32)

    mask = np.tril(np.ones((H, H), np.float32), -1)
    w_r = v_r * mask
    A = w_r - w_r.T                                           # [H, H]
    # A_sb[p, (k*MT+m)*128 + c] = A[k*128+p, m*128+c]
    A_sb = np.ascontiguousarray(
        A.reshape(KT, 128, MT, 128).transpose(1, 0, 2, 3).reshape(128, KT * MT * 128)
    ).astype(ml_dtypes.bfloat16)
    # Wo_sb[p, k*D + d] = w_o[d, k*128+p]   (w_o^T tiles, moving operand)
    Wo_sb = np.ascontiguousarray(
        w_o.T.reshape(KT, 128, D).transpose(1, 0, 2).reshape(128, KT * D)
    ).astype(ml_dtypes.bfloat16)
    # Br[p, m*BS+b] = b_r[m*128+p]
    Br = np.ascontiguousarray(
        np.broadcast_to(b_r.reshape(MT, 128, 1).transpose(1, 0, 2), (128, MT, BS))
    ).reshape(128, MT * BS).astype(np.float32)

    in_maps = []
    for c in range(NCORES):
        hc = h_init[c * BS:(c + 1) * BS]                       # [BS, H]
        h0 = np.ascontiguousarray(
            hc.reshape(BS, MT, 128).transpose(2, 1, 0)         # [128, MT, BS]
        ).reshape(128, MT * BS).astype(np.float32)
        th0 = np.tanh(h0)
        in_maps.append({
            "A": A_sb, "Wo": Wo_sb, "Br": Br,
            "h0": h0, "th0": th0.astype(ml_dtypes.bfloat16),
            "x": np.ascontiguousarray(x[:, c * BS:(c + 1) * BS, :] - b_o),
        })
    return in_maps


def kernel(x, h_init, v_r, b_r, w_o, b_o):
    global _BUILT, LAST_RESULTS
    if _BUILT is None:
        _BUILT = _build_bass()
    nc = _BUILT
    in_maps = _host_prep(x, h_init, v_r, b_r, w_o, b_o)
    res = run_bass_kernel_spmd(nc, in_maps, core_ids=list(range(NCORES)),
                               trace=TRACE)
    LAST_RESULTS = res
    out = np.empty((S, B, D), np.float32)
    for c in range(NCORES):
        out[:, c * BS:(c + 1) * BS, :] = np.asarray(res.results[c]["err"])
    return out
